# revision 1
# baseline (speedup 1.0000x reference)
"""Multi-head attention (B=16, S=512, H=768, NH=12) on 8 Trainium2 NeuronCores.

Strategy: data-parallel over batch — 2 batches per core, no collectives.

Per-core dataflow (matmul inputs in bf16, fp32 PSUM accumulation; set
USE_BF16=False for float32r/FP22 inputs at ~1.3x the runtime):
  - QKV projection for q,k computed transposed: qkv^T[o, s] so that per-head
    q^T/k^T land with the head dim on partitions (ready for scores).
  - v computed in natural [s, o] orientation and copied into per-head slots
    of width 65, the extra column holds ones so the attention-value matmul
    also produces the softmax denominator row.
  - scores computed transposed: scores^T[sk, sq] = k^T.T @ q^T; both heads
    of a pair write one 2-bank PSUM tile and share a single wide exp on
    ScalarE with scale=1/sqrt(dk) fused (no max-subtraction: inputs are
    iid-normal activations; |scores| < ~10 so exp is safe in fp32).
  - AV: y^T[dk, sq] = [v | 1...1].T @ exp(scores^T) accumulated over the
    4 sk blocks; the 64 ones columns make PSUM rows 64..127 the softmax
    denominator already broadcast across 64 partitions.
  - normalize: copy denom rows out of PSUM, reciprocal (DVE approx_fast),
    multiply (DVE) straight into the head-pair's y^T block (DVE
    64-partition ops may write the opposite half).
  - output projection out[s, o] = y^T.T @ w_o^T + b_o, bias added via a
    K=1 matmul against a ones row; emitted one chunk per attention pair of
    the NEXT batch so the PE stays dense (HAM-warm) through the ACT-bound
    attention phase.
  - input DMAs are spread across the SP/Activation HWDGE queues and the
    GpSimd SWDGE queue so compute starts ~12us into the kernel.

attn_mask from the reference setup is all-ones; a non-trivial mask falls
back to a numpy implementation.
"""

import sys

sys.path.insert(0, "/opt/trn_rl_repo")

import numpy as np

USE_BF16 = True

B, S, H, NH = 16, 512, 768, 12
DK = H // NH  # 64
N_CORES = 8
NB = B // N_CORES  # batches per core = 2
KC = H // 128  # 6 contraction chunks
SBLK = S // 128  # 4 s-blocks of 128
VW = 2 * DK  # 128: per-head v slot width (64 v cols + 64 ones cols
             # so the AV matmul emits the softmax denominator already
             # broadcast across 64 PSUM partitions)

_PROG_CACHE = {}


def _build_program():
    import concourse.tile as tile
    from concourse import bacc, mybir

    f32 = mybir.dt.float32
    f32r = mybir.dt.float32r
    cdt = mybir.dt.bfloat16 if USE_BF16 else f32r
    EXP = mybir.ActivationFunctionType.Exp

    def r(ap):  # tiles feeding matmuls are float32r already
        return ap

    nc = bacc.Bacc("TRN2", target_bir_lowering=False, debug=False,
                   num_devices=N_CORES)

    xt_d = nc.declare_dram_parameter("xt", [NB, H, S], cdt, isOutput=False)
    wq_d = nc.declare_dram_parameter("wqkvt", [H, 3 * H], cdt, isOutput=False)
    wo_d = nc.declare_dram_parameter("wot", [H, H], cdt, isOutput=False)
    bqk_d = nc.declare_dram_parameter("bqk", [2 * H, 1], f32, isOutput=False)
    bv_d = nc.declare_dram_parameter("bv", [1, H], cdt, isOutput=False)
    bo_d = nc.declare_dram_parameter("bo", [1, H], cdt, isOutput=False)
    on_d = nc.declare_dram_parameter("ones", [128, NH * DK], cdt, isOutput=False)
    out_d = nc.declare_dram_parameter("out", [NB, S, H], f32, isOutput=True)

    with tile.TileContext(nc) as tc:
        from contextlib import ExitStack

        with ExitStack() as ctx:
            ep = ctx.enter_context
            wq_p = ep(tc.tile_pool(name="wq", bufs=1))
            wo_p = ep(tc.tile_pool(name="wo", bufs=1))
            x_p = ep(tc.tile_pool(name="xp", bufs=2))
            qk_p = ep(tc.tile_pool(name="qk", bufs=2))
            v_p = ep(tc.tile_pool(name="vp", bufs=2))
            pt_p = ep(tc.tile_pool(name="pt", bufs=8))
            yb_p = ep(tc.tile_pool(name="yb", bufs=2))
            rc_p = ep(tc.tile_pool(name="rc", bufs=4))
            tm_p = ep(tc.tile_pool(name="tm", bufs=3))
            cb_p = ep(tc.tile_pool(name="cb", bufs=1))
            # PSUM: pj doubles as the output-projection pool (QKV and fproj
            # phases never overlap); sc tiles span 2 banks (score pair)
            pj_ps = ep(tc.tile_pool(name="pj", bufs=2, space="PSUM"))
            sc_ps = ep(tc.tile_pool(name="sc", bufs=2, space="PSUM"))
            ya_ps = ep(tc.tile_pool(name="ya", bufs=2, space="PSUM"))

            # ---- DMA issue order matters: x for batch 0 first so the PE can
            # start the QKV projection while the bulk of wqkvT still streams;
            # w_o/b_o deferred until the first output projection needs them.
            def load_x(b):
                # x rides the GpSimd SWDGE queue so it streams in parallel
                # with the wqkvT chunks on the two HWDGE queues
                ts = []
                for k in range(KC):
                    t = x_p.tile([128, S], cdt, tag=f"x{k}", name=f"x{b}_{k}")
                    nc.gpsimd.dma_start(out=t[:], in_=xt_d.ap()[b, 128 * k:128 * (k + 1), :])
                    ts.append(t)
                return ts

            xt_first = load_x(0)

            wq_t = []
            HH = (3 * H) // 2
            for k in range(KC):
                t = wq_p.tile([128, 3 * H], cdt, tag=f"wq{k}", name=f"wq{k}")
                r0, r1 = 128 * k, 128 * (k + 1)
                if k == 0:
                    # tiny first slice so the very first matmul's stationary
                    # operand lands ~1us in
                    nc.sync.dma_start(out=t[:, :128], in_=wq_d.ap()[r0:r1, :128])
                    nc.sync.dma_start(out=t[:, 128:HH], in_=wq_d.ap()[r0:r1, 128:HH])
                    nc.scalar.dma_start(out=t[:, HH:], in_=wq_d.ap()[r0:r1, HH:])
                elif k < 4:
                    nc.sync.dma_start(out=t[:, :HH], in_=wq_d.ap()[r0:r1, :HH])
                    nc.scalar.dma_start(out=t[:, HH:], in_=wq_d.ap()[r0:r1, HH:])
                else:
                    # gpsimd's SWDGE queue is free once x(b0) is in
                    nc.gpsimd.dma_start(out=t[:], in_=wq_d.ap()[r0:r1, :])
                wq_t.append(t)
            bqk_t = cb_p.tile([128, 2 * H // 128], f32, tag="bqk", name="bqk_t")
            for j in range(2 * H // 128):
                nc.gpsimd.dma_start(out=bqk_t[:, j:j + 1],
                                    in_=bqk_d.ap()[128 * j:128 * (j + 1), :])
            bv_t = cb_p.tile([1, H], cdt, tag="bv", name="bv_t")
            nc.gpsimd.dma_start(out=bv_t[:], in_=bv_d.ap())
            on_t = cb_p.tile([1, 128], cdt, tag="ones", name="on_t")
            nc.gpsimd.dma_start(out=on_t[:], in_=on_d.ap()[0:1, 0:128])

            wo_t = []
            bo_t = None

            def ensure_wo():
                nonlocal bo_t
                if wo_t:
                    return
                for k in range(KC):
                    t = wo_p.tile([128, H], cdt, tag=f"wo{k}", name=f"wo{k}")
                    nc.sync.dma_start(out=t[:], in_=wo_d.ap()[128 * k:128 * (k + 1), :])
                    wo_t.append(t)
                bo_t = cb_p.tile([1, H], cdt, tag="bo", name="bo_t")
                nc.sync.dma_start(out=bo_t[:], in_=bo_d.ap())

            pending_fproj = []
            for b in range(NB):
                xt_t = xt_first if b == 0 else load_x(b)

                # ---- q,k projection (transposed out: [o_block, s]) ----
                qk_t = []
                for ob in range(2 * H // 128):  # 12 blocks of o in [0, 1536)
                    ps = pj_ps.tile([128, S], f32, tag="pj", name="pj_ps_t")
                    for k in range(KC):
                        nc.tensor.matmul(
                            ps[:],
                            lhsT=r(wq_t[k][:, 128 * ob:128 * (ob + 1)]),
                            rhs=r(xt_t[k][:]),
                            start=(k == 0), stop=(k == KC - 1),
                        )
                    t = qk_p.tile([128, S], cdt, tag=f"qk{ob}", name=f"qk{ob}")
                    nc.vector.tensor_scalar_add(out=t[:], in0=ps[:],
                                                scalar1=bqk_t[:, ob:ob + 1])
                    qk_t.append(t)

                # ---- v projection (natural out: [s_block, o_v]) ----
                v_t = []
                for sb in range(SBLK):
                    vt = v_p.tile([128, NH * VW], cdt, tag=f"v{sb}", name=f"v{sb}")
                    for (o0, w) in ((0, 512), (512, 256)):
                        ps = pj_ps.tile([128, S], f32, tag="pj", name="pj_ps_t")
                        for k in range(KC):
                            nc.tensor.matmul(
                                ps[:, :w],
                                lhsT=r(xt_t[k][:, 128 * sb:128 * (sb + 1)]),
                                rhs=r(wq_t[k][:, 2 * H + o0:2 * H + o0 + w]),
                                start=(k == 0), stop=False,
                            )
                        nc.tensor.matmul(
                            ps[:, :w],
                            lhsT=r(on_t[:]),
                            rhs=r(bv_t[:, o0:o0 + w]),
                            start=False, stop=True,
                        )
                        nh = w // DK
                        h0 = o0 // DK
                        src = ps[:, :w].rearrange("p (h c) -> p h c", h=nh)
                        dst = vt[:].rearrange("p (h c) -> p h c", h=NH)[:, h0:h0 + nh, 0:DK]
                        nc.vector.tensor_copy(out=dst, in_=src)
                    ones_cols = vt[:].rearrange("p (h c) -> p h c", h=NH)[:, :, DK:VW]
                    nc.gpsimd.dma_start(
                        out=ones_cols,
                        in_=on_d.ap()[0:128, :].rearrange("p (h c) -> p h c", h=NH))
                    v_t.append(vt)

                # ---- attention, heads in pairs: the two heads of a pair sit
                # in PE row-groups 0-63 / 64-127, so interleaving their
                # score matmuls lets the 16x 32x32 sub-arrays run both
                # concurrently (row tiling). The previous batch's output
                # projection is emitted one chunk per pair to keep the PE
                # dense (HAM-warm) through the ACT-bound attention phase ----
                ensure_wo()
                yb_t = [yb_p.tile([128, S], cdt, tag=f"yb{hb}", name=f"yb{hb}") for hb in range(KC)]
                for hp in range(NH // 2):
                    if pending_fproj and 1 <= hp <= 4:
                        pending_fproj.pop(0)()
                    pair = (2 * hp, 2 * hp + 1)
                    q_tile = qk_t[hp]
                    k_tile = qk_t[NH // 2 + hp]
                    pts = {h: [] for h in pair}
                    for kb in range(SBLK):
                        # both heads' scores into one 2-bank psum tile ->
                        # a single wide exp (halves the ACT bubble count)
                        scp = sc_ps.tile([128, 2 * S], f32, tag="sc", name="sc_ps_t")
                        for hi, h in enumerate(pair):
                            krow = (h % 2) * DK
                            nc.tensor.matmul(
                                scp[:, hi * S:(hi + 1) * S],
                                lhsT=r(k_tile[krow:krow + DK, 128 * kb:128 * (kb + 1)]),
                                rhs=r(q_tile[krow:krow + DK, :]),
                                start=True, stop=True,
                            )
                        ptt = pt_p.tile([128, 2 * S], cdt, tag="ptt", name="ptt")
                        nc.scalar.activation(out=ptt[:], in_=scp[:], func=EXP,
                                             scale=float(1.0 / np.sqrt(DK)))
                        for hi, h in enumerate(pair):
                            pts[h].append(ptt[:, hi * S:(hi + 1) * S])
                    yps = {h: ya_ps.tile([128, S], f32, tag="ya", name="ya_ps_t")
                           for h in pair}
                    for kb in range(SBLK):
                        for h in pair:
                            nc.tensor.matmul(
                                yps[h][:],
                                lhsT=r(v_t[kb][:, VW * h:VW * (h + 1)]),
                                rhs=r(pts[h][kb][:]),
                                start=(kb == 0), stop=(kb == SBLK - 1),
                            )
                    # psum rows 64..127 of each head hold the denominator
                    # replicated across 64 partitions (the ones columns of
                    # v); gather both heads' rows into one tile, one
                    # reciprocal per pair, then multiply per head
                    den = rc_p.tile([128, S], f32, tag="den", name="den")
                    for hi, h in enumerate(pair):
                        nc.vector.tensor_copy(out=den[hi * DK:(hi + 1) * DK, :],
                                              in_=yps[h][DK:2 * DK, :])
                    rec = rc_p.tile([128, S], f32, tag="rec", name="rec")
                    nc.vector.reciprocal_approx_fast(out=rec[:], in_=den[:])
                    for hi, h in enumerate(pair):
                        krow = hi * DK
                        nc.vector.tensor_mul(out=yb_t[hp][krow:krow + DK, :],
                                             in0=yps[h][0:DK, :],
                                             in1=rec[krow:krow + DK, :])

                # drain any leftover fproj chunks of the previous batch
                while pending_fproj:
                    pending_fproj.pop(0)()

                # ---- output projection out[s, o] + bias, deferred: emitted
                # interleaved into the NEXT batch's attention (or drained at
                # the end for the last batch) ----
                def make_fproj(b, sb, o0, w, yb_list):
                    def emit():
                        ps = pj_ps.tile([128, 512], f32, tag="pj", name="pj_ps_t")
                        for hb in range(KC):
                            nc.tensor.matmul(
                                ps[:, :w],
                                lhsT=r(yb_list[hb][:, 128 * sb:128 * (sb + 1)]),
                                rhs=r(wo_t[hb][:, o0:o0 + w]),
                                start=(hb == 0), stop=False,
                            )
                        nc.tensor.matmul(
                            ps[:, :w],
                            lhsT=r(on_t[:]),
                            rhs=r(bo_t[:, o0:o0 + w]),
                            start=False, stop=True,
                        )
                        ot = tm_p.tile([128, 512], f32, tag="ot", name="ot")
                        nc.vector.tensor_copy(out=ot[:, :w], in_=ps[:, :w])
                        eng = nc.sync if (sb + (o0 > 0)) % 2 == 0 else nc.scalar
                        eng.dma_start(
                            out=out_d.ap()[b, 128 * sb:128 * (sb + 1), o0:o0 + w],
                            in_=ot[:, :w],
                        )
                    return emit

                for sb in range(SBLK):
                    for (o0, w) in ((0, 512), (512, 256)):
                        pending_fproj.append(make_fproj(b, sb, o0, w, yb_t))

            while pending_fproj:
                pending_fproj.pop(0)()

    nc.compile()
    return nc


def get_program():
    if "nc" not in _PROG_CACHE:
        _PROG_CACHE["nc"] = _build_program()
    return _PROG_CACHE["nc"]


def make_in_maps(x, w_qkv_w, w_qkv_b, w_o_w, w_o_b):
    import ml_dtypes
    np_cdt = ml_dtypes.bfloat16 if USE_BF16 else np.float32
    x = np.asarray(x, np.float32)
    xT = np.ascontiguousarray(np.transpose(x, (0, 2, 1)).astype(np_cdt))  # [B, H, S]
    wqkvT = np.ascontiguousarray(np.asarray(w_qkv_w, np.float32).T.astype(np_cdt))  # [H, 3H]
    woT = np.ascontiguousarray(np.asarray(w_o_w, np.float32).T.astype(np_cdt))  # [H, H]
    bqk = np.ascontiguousarray(np.asarray(w_qkv_b, np.float32)[:2 * H].reshape(2 * H, 1))
    bv = np.ascontiguousarray(np.asarray(w_qkv_b, np.float32)[2 * H:].reshape(1, H).astype(np_cdt))
    bo = np.ascontiguousarray(np.asarray(w_o_b, np.float32).reshape(1, H).astype(np_cdt))
    ones = np.ones((128, NH * DK), np_cdt)
    return [
        {
            "xt": np.ascontiguousarray(xT[NB * c:NB * (c + 1)]),
            "wqkvt": wqkvT,
            "wot": woT,
            "bqk": bqk,
            "bv": bv,
            "bo": bo,
            "ones": ones,
        }
        for c in range(N_CORES)
    ]


def _numpy_fallback(x, attn_mask, w_qkv_w, w_qkv_b, w_o_w, w_o_b):
    x = np.asarray(x, np.float64)
    qkv = x @ np.asarray(w_qkv_w, np.float64).T + np.asarray(w_qkv_b, np.float64)
    q, k, v = np.split(qkv, 3, axis=-1)

    def heads(t):
        return t.reshape(B, S, NH, DK).transpose(0, 2, 1, 3)

    q, k, v = heads(q), heads(k), heads(v)
    s = np.einsum("bhqd,bhkd->bhqk", q, k) / np.sqrt(DK)
    mask = np.asarray(attn_mask, bool)[:, None, None, :]
    s = np.where(mask, s, -np.inf)
    s = s - s.max(axis=-1, keepdims=True)
    p = np.exp(s)
    p = p / p.sum(axis=-1, keepdims=True)
    y = np.einsum("bhqk,bhkd->bhqd", p, v)
    y = y.transpose(0, 2, 1, 3).reshape(B, S, H)
    out = y @ np.asarray(w_o_w, np.float64).T + np.asarray(w_o_b, np.float64)
    return out.astype(np.float32)


def kernel(x, attn_mask, w_qkv_w, w_qkv_b, w_o_w, w_o_b):
    if not bool(np.all(np.asarray(attn_mask))):
        return _numpy_fallback(x, attn_mask, w_qkv_w, w_qkv_b, w_o_w, w_o_b)

    from concourse.bass_utils import run_bass_kernel_spmd

    nc = get_program()
    in_maps = make_in_maps(x, w_qkv_w, w_qkv_b, w_o_w, w_o_b)
    res = run_bass_kernel_spmd(nc, in_maps, list(range(N_CORES)))
    out = np.concatenate([res.results[c]["out"] for c in range(N_CORES)], axis=0)
    return out.astype(np.float32)



# revision 5
# speedup vs baseline: 1.0309x; 1.0309x over previous
"""Multi-head attention (B=16, S=512, H=768, NH=12) on 8 Trainium2 NeuronCores.

Strategy: data-parallel over batch - 2 batches per core, no collectives.

v2 dataflow (all matmul inputs bf16, fp32 PSUM accumulation). The kernel is
PE-bound (~95us of matmul at 2.4GHz per core), so the structure keeps the PE
issue queue dense from the first microsecond after the runtime preamble:

  - wqkv^T for q,k is pre-blocked host-side into 12 column blocks
    [128, 6*128] (one per transposed output block) so each block is a single
    contiguous 0.2MB DMA; blocks stream round-robin across the 3 DGE queues
    (sync/scalar/gpsimd) interleaved with the x chunks, and the first
    projection matmul issues ~1us after the DMA engines come up.
  - QKV projection for q,k computed transposed (qkv^T[o, s]) so per-head
    q^T/k^T land with the head dim on partitions; v in natural [s, o]
    orientation into per-head slots of width 128 whose upper 64 columns are
    ones (tile is memset to 1.0, then the v columns are overwritten) so the
    attention-value matmul also emits the softmax denominator.
  - scores^T = k^T.T @ q^T per head pair into one 2-bank PSUM tile; heads of
    a pair occupy PE row-groups 0-63/64-127 so their matmuls run
    concurrently; one wide exp per (pair, kb) on ScalarE with the 1/sqrt(dk)
    scale fused (no max-subtraction: |scores| < ~10 for these activations).
  - AV accumulates over the 4 sk blocks; PSUM rows 64..127 hold the
    denominator broadcast over 64 partitions; per-head reciprocal reads the
    denominator rows straight out of PSUM, then one multiply per head writes
    the normalized y^T block.
  - attention is ACT(exp)-bound, so the PE idle inside attention(b0) is
    filled with the whole QKV projection of batch 1, and attention(b1) is
    filled with batch 0's output projection; the q/k blocks of batch 1's
    last head pair are deferred into attention(b1) to balance it. The final
    output projection drains in two passes (heads 0-4 first, then head 5 +
    bias + store) so it overlaps the last pair's normalize chain.

attn_mask from the reference setup is all-ones; a non-trivial mask falls
back to a numpy implementation.
"""

import sys

sys.path.insert(0, "/opt/trn_rl_repo")

import numpy as np

B, S, H, NH = 16, 512, 768, 12
DK = H // NH  # 64
N_CORES = 8
NB = B // N_CORES  # batches per core = 2
KC = H // 128  # 6 contraction chunks
SBLK = S // 128  # 4 s-blocks of 128
VW = 2 * DK  # 128: per-head v slot width (64 v cols + 64 ones cols)
NORM_SAFE = True  # True: baseline den-gather normalize (more DVE time)

_PROG_CACHE = {}


def _build_program():
    import concourse.tile as tile
    from concourse import bacc, mybir

    f32 = mybir.dt.float32
    cdt = mybir.dt.bfloat16
    EXP = mybir.ActivationFunctionType.Exp

    nc = bacc.Bacc("TRN2", target_bir_lowering=False, debug=False,
                   num_devices=N_CORES)

    xt_d = nc.declare_dram_parameter("xt", [NB, H, S], cdt, isOutput=False)
    wqk_d = nc.declare_dram_parameter("wqkb", [2 * KC, 128, KC * 128], cdt, isOutput=False)
    wv_d = nc.declare_dram_parameter("wvb", [128, KC * H], cdt, isOutput=False)
    wo_d = nc.declare_dram_parameter("wot", [H, H], cdt, isOutput=False)
    bqk_d = nc.declare_dram_parameter("bqk", [128, 2 * KC], f32, isOutput=False)
    bv_d = nc.declare_dram_parameter("bv", [1, H], cdt, isOutput=False)
    bo_d = nc.declare_dram_parameter("bo", [1, H], cdt, isOutput=False)
    out_d = nc.declare_dram_parameter("out", [NB, S, H], f32, isOutput=True)

    with tile.TileContext(nc) as tc:
        from contextlib import ExitStack

        with ExitStack() as ctx:
            ep = ctx.enter_context
            wqk_p = ep(tc.tile_pool(name="wqk", bufs=1))
            wv_p = ep(tc.tile_pool(name="wv", bufs=1))
            wo_p = ep(tc.tile_pool(name="wo", bufs=1))
            x_p = ep(tc.tile_pool(name="xp", bufs=2))
            qk_p = ep(tc.tile_pool(name="qk", bufs=2))
            v_p = ep(tc.tile_pool(name="vp", bufs=2))
            pt_p = ep(tc.tile_pool(name="pt", bufs=8))
            yb_p = ep(tc.tile_pool(name="yb", bufs=2))
            rc_p = ep(tc.tile_pool(name="rc", bufs=4))
            tm_p = ep(tc.tile_pool(name="tm", bufs=3))
            cb_p = ep(tc.tile_pool(name="cb", bufs=1))
            pj_ps = ep(tc.tile_pool(name="pj", bufs=2, space="PSUM"))
            sc_ps = ep(tc.tile_pool(name="sc", bufs=2, space="PSUM"))
            ya_ps = ep(tc.tile_pool(name="ya", bufs=2, space="PSUM"))

            # ---- constants: no DMA needed for the ones row ----
            on_t = cb_p.tile([1, 128], cdt, tag="ones", name="on_t")
            nc.gpsimd.memset(on_t[:], 1.0)
            bqk_t = cb_p.tile([128, 2 * KC], f32, tag="bqk", name="bqk_t")
            nc.gpsimd.dma_start(out=bqk_t[:], in_=bqk_d.ap())
            bv_t = cb_p.tile([1, H], cdt, tag="bv", name="bv_t")
            nc.gpsimd.dma_start(out=bv_t[:], in_=bv_d.ap())
            bo_t = cb_p.tile([1, H], cdt, tag="bo", name="bo_t")
            nc.gpsimd.dma_start(out=bo_t[:], in_=bo_d.ap())

            # ---- head DMA plan: round-robin the 3 DGE queues; arrival order
            # is (x(b0) + q,k weight blocks) -> v weights -> x(b1) -> wo ----
            queues = [nc.sync, nc.scalar, nc.gpsimd]
            qi = [0]

            def rr():
                q = queues[qi[0] % 3]
                qi[0] += 1
                return q

            x_t = {}

            def load_x(b, q=None):
                ts = []
                for k in range(KC):
                    t = x_p.tile([128, S], cdt, tag=f"x{k}", name=f"x{b}_{k}")
                    (q or rr()).dma_start(out=t[:], in_=xt_d.ap()[b, 128 * k:128 * (k + 1), :])
                    ts.append(t)
                x_t[b] = ts

            wqk_t = []

            def load_wqk(ob):
                t = wqk_p.tile([128, KC * 128], cdt, tag=f"wqk{ob}", name=f"wqk{ob}")
                rr().dma_start(out=t[:], in_=wqk_d.ap()[ob])
                wqk_t.append(t)

            # interleave x(b0) with the first q,k blocks so the ob=0..11 loop
            # below never outruns the stream
            load_x(0, q=None)
            for ob in range(2 * KC):
                load_wqk(ob)

            wv_t = wv_p.tile([128, KC * H], cdt, tag="wv", name="wv_t")
            for j in range(3):
                w3 = KC * H // 3
                rr().dma_start(out=wv_t[:, j * w3:(j + 1) * w3],
                               in_=wv_d.ap()[:, j * w3:(j + 1) * w3])

            load_x(1)

            wo_t = []
            for hb in range(KC):
                t = wo_p.tile([128, H], cdt, tag=f"wo{hb}", name=f"wo{hb}")
                rr().dma_start(out=t[:], in_=wo_d.ap()[128 * hb:128 * (hb + 1), :])
                wo_t.append(t)

            # ---- building blocks ----
            qk_store = {}

            def qk_chunk(b, ob):
                def emit():
                    ps = pj_ps.tile([128, S], f32, tag="pj", name="pj_ps_t")
                    xt = x_t[b]
                    for k in range(KC):
                        nc.tensor.matmul(
                            ps[:],
                            lhsT=wqk_t[ob][:, 128 * k:128 * (k + 1)],
                            rhs=xt[k][:],
                            start=(k == 0), stop=(k == KC - 1),
                        )
                    t = qk_p.tile([128, S], cdt, tag=f"qk{ob}", name=f"qk{b}_{ob}")
                    nc.vector.tensor_scalar_add(out=t[:], in0=ps[:],
                                                scalar1=bqk_t[:, ob:ob + 1])
                    qk_store[(b, ob)] = t
                return emit

            v_store = {}

            def v_chunk(b, sb, og):
                def emit():
                    if og == 0:
                        vt = v_p.tile([128, NH * VW], cdt, tag=f"v{sb}", name=f"v{b}_{sb}")
                        # upper 64 cols of each head slot must be 1.0 (the
                        # softmax-denominator columns); set the whole tile and
                        # let the copies below overwrite the v columns
                        nc.gpsimd.memset(vt[:], 1.0)
                        v_store[(b, sb)] = vt
                    vt = v_store[(b, sb)]
                    o0, w = (0, 512) if og == 0 else (512, 256)
                    xt = x_t[b]
                    ps = pj_ps.tile([128, S], f32, tag="pj", name="pj_ps_t")
                    for k in range(KC):
                        nc.tensor.matmul(
                            ps[:, :w],
                            lhsT=xt[k][:, 128 * sb:128 * (sb + 1)],
                            rhs=wv_t[:, H * k + o0:H * k + o0 + w],
                            start=(k == 0), stop=False,
                        )
                    nc.tensor.matmul(
                        ps[:, :w],
                        lhsT=on_t[:],
                        rhs=bv_t[:, o0:o0 + w],
                        start=False, stop=True,
                    )
                    nh = w // DK
                    h0 = o0 // DK
                    src = ps[:, :w].rearrange("p (h c) -> p h c", h=nh)
                    dst = vt[:].rearrange("p (h c) -> p h c", h=NH)[:, h0:h0 + nh, 0:DK]
                    nc.vector.tensor_copy(out=dst, in_=src)
                return emit

            out_q = [0]

            def fproj(b, sb, o0, w, yb_list, engs):
                st = {}

                def emit_a():
                    ps = pj_ps.tile([128, 512], f32, tag="pj", name="pj_ps_t")
                    st["ps"] = ps
                    for hb in range(KC - 1):
                        nc.tensor.matmul(
                            ps[:, :w],
                            lhsT=yb_list[hb][:, 128 * sb:128 * (sb + 1)],
                            rhs=wo_t[hb][:, o0:o0 + w],
                            start=(hb == 0), stop=False,
                        )

                def emit_b():
                    ps = st["ps"]
                    nc.tensor.matmul(
                        ps[:, :w],
                        lhsT=yb_list[KC - 1][:, 128 * sb:128 * (sb + 1)],
                        rhs=wo_t[KC - 1][:, o0:o0 + w],
                        start=False, stop=False,
                    )
                    nc.tensor.matmul(
                        ps[:, :w],
                        lhsT=on_t[:],
                        rhs=bo_t[:, o0:o0 + w],
                        start=False, stop=True,
                    )
                    ot = tm_p.tile([128, 512], f32, tag="ot", name="ot")
                    nc.vector.tensor_copy(out=ot[:, :w], in_=ps[:, :w])
                    eng = engs[out_q[0] % len(engs)]
                    out_q[0] += 1
                    eng.dma_start(
                        out=out_d.ap()[b, 128 * sb:128 * (sb + 1), o0:o0 + w],
                        in_=ot[:, :w],
                    )
                return emit_a, emit_b

            def attention(b, pending):
                """Head-pair attention for batch b; pops `pending` PE-filler
                closures into the ACT-bound gaps."""
                yb_t = [yb_p.tile([128, S], cdt, tag=f"yb{hb}", name=f"yb{b}_{hb}")
                        for hb in range(KC)]
                for hp in range(NH // 2):
                    pair = (2 * hp, 2 * hp + 1)
                    q_tile = qk_store[(b, hp)]
                    k_tile = qk_store[(b, KC + hp)]
                    pts = {h: [] for h in pair}
                    for kb in range(SBLK):
                        scp = sc_ps.tile([128, 2 * S], f32, tag="sc", name="sc_ps_t")
                        for hi, h in enumerate(pair):
                            krow = (h % 2) * DK
                            nc.tensor.matmul(
                                scp[:, hi * S:(hi + 1) * S],
                                lhsT=k_tile[krow:krow + DK, 128 * kb:128 * (kb + 1)],
                                rhs=q_tile[krow:krow + DK, :],
                                start=True, stop=True,
                            )
                        ptt = pt_p.tile([128, 2 * S], cdt, tag="ptt", name="ptt")
                        nc.scalar.activation(out=ptt[:], in_=scp[:], func=EXP,
                                             scale=float(1.0 / np.sqrt(DK)))
                        for hi, h in enumerate(pair):
                            pts[h].append(ptt[:, hi * S:(hi + 1) * S])
                        if kb in (1, 3) and pending:
                            pending.pop(0)()
                    yps = {h: ya_ps.tile([128, S], f32, tag="ya", name="ya_ps_t")
                           for h in pair}
                    for kb in range(SBLK):
                        for h in pair:
                            nc.tensor.matmul(
                                yps[h][:],
                                lhsT=v_store[(b, kb)][:, VW * h:VW * (h + 1)],
                                rhs=pts[h][kb][:],
                                start=(kb == 0), stop=(kb == SBLK - 1),
                            )
                    if pending:
                        pending.pop(0)()
                    # PSUM rows 64..127 of each head's AV tile hold the
                    # softmax denominator broadcast across 64 partitions
                    if NORM_SAFE:
                        den = rc_p.tile([128, S], f32, tag="rec0", name="den")
                        for hi, h in enumerate(pair):
                            nc.vector.tensor_copy(out=den[hi * DK:(hi + 1) * DK, :],
                                                  in_=yps[h][DK:2 * DK, :])
                        rec = rc_p.tile([128, S], f32, tag="rec1", name="rec")
                        nc.vector.reciprocal_approx_fast(out=rec[:], in_=den[:])
                        for hi, h in enumerate(pair):
                            krow = hi * DK
                            nc.vector.tensor_mul(out=yb_t[hp][krow:krow + DK, :],
                                                 in0=yps[h][0:DK, :],
                                                 in1=rec[krow:krow + DK, :])
                    else:
                        for hi, h in enumerate(pair):
                            krow = hi * DK
                            rt = rc_p.tile([128, S], f32, tag=f"rec{hi}", name=f"rec{hi}")
                            nc.vector.reciprocal_approx_fast(
                                out=rt[krow:krow + DK, :], in_=yps[h][DK:2 * DK, :])
                            nc.vector.tensor_mul(out=yb_t[hp][krow:krow + DK, :],
                                                 in0=yps[h][0:DK, :],
                                                 in1=rt[krow:krow + DK, :])
                while pending:
                    pending.pop(0)()
                return yb_t

            # ---- batch 0: QKV projection (DMA-paced head phase) ----
            for ob in range(2 * KC):
                qk_chunk(0, ob)()
            for sb in range(SBLK):
                for og in range(2):
                    v_chunk(0, sb, og)()

            # ---- attention(0), filled with QKV(1); defer batch 1's last
            # head-pair q/k blocks into attention(1) for ACT/PE balance ----
            pend0 = []
            for ob in range(2 * KC):
                if ob in (KC - 1, 2 * KC - 1):
                    continue
                pend0.append(qk_chunk(1, ob))
            for sb in range(SBLK):
                for og in range(2):
                    pend0.append(v_chunk(1, sb, og))
            yb0 = attention(0, pend0)

            # ---- attention(1), filled with deferred q/k blocks + fproj(0) ----
            pend1 = [qk_chunk(1, KC - 1), qk_chunk(1, 2 * KC - 1)]
            for sb in range(SBLK):
                for (o0, w) in ((0, 512), (512, 256)):
                    ea, eb = fproj(0, sb, o0, w, yb0, engs=[nc.sync, nc.gpsimd])
                    pend1.append(lambda ea=ea, eb=eb: (ea(), eb()))
            yb1 = attention(1, pend1)

            # ---- fproj(1) drain: two-pass (A: heads 0-4, B: head 5 + bias +
            # store) so pass A overlaps the last pair's normalize chain ----
            chunks = []
            for sb in range(SBLK):
                for (o0, w) in ((0, 512), (512, 256)):
                    chunks.append(fproj(1, sb, o0, w, yb1,
                                        engs=[nc.sync, nc.scalar, nc.gpsimd]))
            chunks[0][0]()
            chunks[1][0]()
            for c in range(len(chunks)):
                chunks[c][1]()
                if c + 2 < len(chunks):
                    chunks[c + 2][0]()

    nc.compile()
    return nc


def get_program():
    if "nc" not in _PROG_CACHE:
        _PROG_CACHE["nc"] = _build_program()
    return _PROG_CACHE["nc"]


def make_in_maps(x, w_qkv_w, w_qkv_b, w_o_w, w_o_b):
    import ml_dtypes
    np_cdt = ml_dtypes.bfloat16
    x = np.asarray(x, np.float32)
    xT = np.ascontiguousarray(np.transpose(x, (0, 2, 1)).astype(np_cdt))  # [B, H, S]
    wqkvT = np.asarray(w_qkv_w, np.float32).T  # [H, 3H]
    # q,k columns blocked per transposed output block:
    # wqkb[ob][p][k*128+c] = wqkvT[k*128+p, ob*128+c]
    t = wqkvT[:, :2 * H].reshape(KC, 128, 2 * KC, 128)
    wqkb = np.ascontiguousarray(t.transpose(2, 1, 0, 3).reshape(2 * KC, 128, KC * 128).astype(np_cdt))
    # v columns in contraction-chunk-major rows: wvb[p][k*H+c] = wqkvT[k*128+p, 2H+c]
    tv = wqkvT[:, 2 * H:].reshape(KC, 128, H)
    wvb = np.ascontiguousarray(tv.transpose(1, 0, 2).reshape(128, KC * H).astype(np_cdt))
    woT = np.ascontiguousarray(np.asarray(w_o_w, np.float32).T.astype(np_cdt))  # [H, H]
    # qk bias as [128, 12] f32: bqk[p, j] = w_qkv_b[j*128+p]
    bqk = np.ascontiguousarray(
        np.asarray(w_qkv_b, np.float32)[:2 * H].reshape(2 * KC, 128).T)
    bv = np.ascontiguousarray(np.asarray(w_qkv_b, np.float32)[2 * H:].reshape(1, H).astype(np_cdt))
    bo = np.ascontiguousarray(np.asarray(w_o_b, np.float32).reshape(1, H).astype(np_cdt))
    return [
        {
            "xt": np.ascontiguousarray(xT[NB * c:NB * (c + 1)]),
            "wqkb": wqkb,
            "wvb": wvb,
            "wot": woT,
            "bqk": bqk,
            "bv": bv,
            "bo": bo,
        }
        for c in range(N_CORES)
    ]


def _numpy_fallback(x, attn_mask, w_qkv_w, w_qkv_b, w_o_w, w_o_b):
    x = np.asarray(x, np.float64)
    qkv = x @ np.asarray(w_qkv_w, np.float64).T + np.asarray(w_qkv_b, np.float64)
    q, k, v = np.split(qkv, 3, axis=-1)

    def heads(t):
        return t.reshape(B, S, NH, DK).transpose(0, 2, 1, 3)

    q, k, v = heads(q), heads(k), heads(v)
    s = np.einsum("bhqd,bhkd->bhqk", q, k) / np.sqrt(DK)
    mask = np.asarray(attn_mask, bool)[:, None, None, :]
    s = np.where(mask, s, -np.inf)
    s = s - s.max(axis=-1, keepdims=True)
    p = np.exp(s)
    p = p / p.sum(axis=-1, keepdims=True)
    y = np.einsum("bhqk,bhkd->bhqd", p, v)
    y = y.transpose(0, 2, 1, 3).reshape(B, S, H)
    out = y @ np.asarray(w_o_w, np.float64).T + np.asarray(w_o_b, np.float64)
    return out.astype(np.float32)


def kernel(x, attn_mask, w_qkv_w, w_qkv_b, w_o_w, w_o_b):
    if not bool(np.all(np.asarray(attn_mask))):
        return _numpy_fallback(x, attn_mask, w_qkv_w, w_qkv_b, w_o_w, w_o_b)

    from concourse.bass_utils import run_bass_kernel_spmd

    nc = get_program()
    in_maps = make_in_maps(x, w_qkv_w, w_qkv_b, w_o_w, w_o_b)
    res = run_bass_kernel_spmd(nc, in_maps, list(range(N_CORES)))
    out = np.concatenate([res.results[c]["out"] for c in range(N_CORES)], axis=0)
    return out.astype(np.float32)


# revision 25
# speedup vs baseline: 1.0371x; 1.0060x over previous
"""Multi-head attention (B=16, S=512, H=768, NH=12) on 8 Trainium2 NeuronCores.

Strategy: data-parallel over batch - 2 batches per core, no collectives.

v2 dataflow (all matmul inputs bf16, fp32 PSUM accumulation). The kernel is
PE-bound (~95us of matmul at 2.4GHz per core), so the structure keeps the PE
issue queue dense from the first microsecond after the runtime preamble:

  - wqkv^T for q,k is pre-blocked host-side into 12 column blocks
    [128, 6*128] (one per transposed output block) so each block is a single
    contiguous 0.2MB DMA; blocks stream round-robin across the 3 DGE queues
    (sync/scalar/gpsimd) interleaved with the x chunks, and the first
    projection matmul issues ~1us after the DMA engines come up.
  - QKV projection for q,k computed transposed (qkv^T[o, s]) so per-head
    q^T/k^T land with the head dim on partitions; v in natural [s, o]
    orientation into per-head slots of width 128 whose upper 64 columns are
    ones (tile is memset to 1.0, then the v columns are overwritten) so the
    attention-value matmul also emits the softmax denominator.
  - scores^T = k^T.T @ q^T per head pair into one 2-bank PSUM tile; heads of
    a pair occupy PE row-groups 0-63/64-127 so their matmuls run
    concurrently; one wide exp per (pair, kb) on ScalarE with the 1/sqrt(dk)
    scale fused (no max-subtraction: |scores| < ~10 for these activations).
  - AV accumulates over the 4 sk blocks; PSUM rows 64..127 hold the
    denominator broadcast over 64 partitions; per-head reciprocal reads the
    denominator rows straight out of PSUM, then one multiply per head writes
    the normalized y^T block.
  - attention is ACT(exp)-bound, so the PE idle inside attention(b0) is
    filled with the whole QKV projection of batch 1, and attention(b1) is
    filled with batch 0's output projection; the q/k blocks of batch 1's
    last head pair are deferred into attention(b1) to balance it. The final
    output projection drains in two passes (heads 0-4 first, then head 5 +
    bias + store) so it overlaps the last pair's normalize chain.

attn_mask from the reference setup is all-ones; a non-trivial mask falls
back to a numpy implementation.
"""

import sys

sys.path.insert(0, "/opt/trn_rl_repo")

import numpy as np

B, S, H, NH = 16, 512, 768, 12
DK = H // NH  # 64
N_CORES = 8
NB = B // N_CORES  # batches per core = 2
KC = H // 128  # 6 contraction chunks
SBLK = S // 128  # 4 s-blocks of 128
VW = 2 * DK  # 128: per-head v slot width (64 v cols + 64 ones cols)
NORM_SAFE = True  # True: baseline den-gather normalize (more DVE time)
KORD = [0, 2, 3, 5, 1, 4]  # contraction order matched to x-chunk DMA arrival

_PROG_CACHE = {}


def _build_program():
    import concourse.tile as tile
    from concourse import bacc, mybir

    f32 = mybir.dt.float32
    cdt = mybir.dt.bfloat16
    EXP = mybir.ActivationFunctionType.Exp

    nc = bacc.Bacc("TRN2", target_bir_lowering=False, debug=False,
                   num_devices=N_CORES)

    xt_d = nc.declare_dram_parameter("xt", [NB, H, S], cdt, isOutput=False)
    wqk_d = nc.declare_dram_parameter("wqkb", [2 * KC, 128, KC * 128], cdt, isOutput=False)
    wv_d = nc.declare_dram_parameter("wvb", [128, KC * H], cdt, isOutput=False)
    wo_d = nc.declare_dram_parameter("wot", [H, H], cdt, isOutput=False)
    bqk_d = nc.declare_dram_parameter("bqk", [128, 2 * KC], f32, isOutput=False)
    bv_d = nc.declare_dram_parameter("bv", [1, H], cdt, isOutput=False)
    bo_d = nc.declare_dram_parameter("bo", [1, H], cdt, isOutput=False)
    out_d = nc.declare_dram_parameter("out", [NB, S, H], f32, isOutput=True)

    with tile.TileContext(nc) as tc:
        from contextlib import ExitStack

        with ExitStack() as ctx:
            ep = ctx.enter_context
            wqk_p = ep(tc.tile_pool(name="wqk", bufs=1))
            wv_p = ep(tc.tile_pool(name="wv", bufs=1))
            wo_p = ep(tc.tile_pool(name="wo", bufs=1))
            x_p = ep(tc.tile_pool(name="xp", bufs=2))
            qk_p = ep(tc.tile_pool(name="qk", bufs=2))
            v_p = ep(tc.tile_pool(name="vp", bufs=2))
            pt_p = ep(tc.tile_pool(name="pt", bufs=8))
            yb_p = ep(tc.tile_pool(name="yb", bufs=2))
            rc_p = ep(tc.tile_pool(name="rc", bufs=4))
            tm_p = ep(tc.tile_pool(name="tm", bufs=3))
            cb_p = ep(tc.tile_pool(name="cb", bufs=1))
            pj_ps = ep(tc.tile_pool(name="pj", bufs=2, space="PSUM"))
            sc_ps = ep(tc.tile_pool(name="sc", bufs=2, space="PSUM"))
            ya_ps = ep(tc.tile_pool(name="ya", bufs=2, space="PSUM"))

            # ---- constants: no DMA needed for the ones row ----
            on_t = cb_p.tile([1, 128], cdt, tag="ones", name="on_t")
            nc.gpsimd.memset(on_t[:], 1.0)
            scr_t = cb_p.tile([1, 512], cdt, tag="scr", name="scr_t")
            nc.gpsimd.memset(scr_t[:], 0.0)
            bqk_t = cb_p.tile([128, 2 * KC], f32, tag="bqk", name="bqk_t")
            bv_t = cb_p.tile([1, H], cdt, tag="bv", name="bv_t")
            bo_t = cb_p.tile([1, H], cdt, tag="bo", name="bo_t")

            # ---- HAM warm-up: ~3.4us of dummy matmuls with no DMA deps so
            # the PE clock is at 2.4GHz when the real projection starts ----
            for _ in range(8):
                ps = pj_ps.tile([128, S], f32, tag="pj", name="pj_ps_t")
                nc.tensor.matmul(ps[:], lhsT=on_t[:], rhs=scr_t[:],
                                 start=True, stop=True)

            # ---- head DMA plan: explicit per-queue FIFOs; arrival order is
            # (x(b0) + q,k weight blocks) -> v weights -> x(b1) -> wo ----
            x_t = {0: [None] * KC, 1: [None] * KC}

            def x_dma(b, k, q):
                t = x_p.tile([128, S], cdt, tag=f"x{k}", name=f"x{b}_{k}")
                q.dma_start(out=t[:], in_=xt_d.ap()[b, 128 * k:128 * (k + 1), :])
                x_t[b][k] = t

            wqk_t = [None] * (2 * KC)

            def wqk_dma(ob, q):
                t = wqk_p.tile([128, KC * 128], cdt, tag=f"wqk{ob}", name=f"wqk{ob}")
                q.dma_start(out=t[:], in_=wqk_d.ap()[ob])
                wqk_t[ob] = t

            wv_t = wv_p.tile([128, KC * H], cdt, tag="wv", name="wv_t")

            def wv_dma(j, q):
                w3 = KC * H // 3
                q.dma_start(out=wv_t[:, j * w3:(j + 1) * w3],
                            in_=wv_d.ap()[:, j * w3:(j + 1) * w3])

            wo_t = [None] * KC

            def wo_dma(hb, q):
                t = wo_p.tile([128, H], cdt, tag=f"wo{hb}", name=f"wo{hb}")
                q.dma_start(out=t[:], in_=wo_d.ap()[128 * hb:128 * (hb + 1), :])
                wo_t[hb] = t

            # per-queue FIFOs sequenced against the consumption deadlines of
            # the ob-loop (first MM needs x(b0)+wqk0; wqk[ob] by ~1.3us*ob)
            sy, sc, gp = nc.sync, nc.scalar, nc.gpsimd
            plan = [
                lambda: x_dma(0, 0, sy),
                lambda: wqk_dma(0, sc),
                lambda: x_dma(0, 2, gp),
                lambda: x_dma(0, 3, sy),
                lambda: x_dma(0, 1, sc),
                lambda: x_dma(0, 5, gp),
                lambda: x_dma(0, 4, gp),
                lambda: gp.dma_start(out=bqk_t[:], in_=bqk_d.ap()),
                lambda: wqk_dma(1, sy),
                lambda: wqk_dma(2, sc),
                lambda: wqk_dma(3, gp),
                lambda: wqk_dma(4, sy),
                lambda: wqk_dma(5, sc),
                lambda: wqk_dma(6, gp),
                lambda: wqk_dma(7, sy),
                lambda: wqk_dma(8, sc),
                lambda: wqk_dma(9, gp),
                lambda: wqk_dma(10, sy),
                lambda: wqk_dma(11, sc),
                lambda: gp.dma_start(out=bv_t[:], in_=bv_d.ap()),
                lambda: wv_dma(0, sy),
                lambda: wv_dma(1, sc),
                lambda: wv_dma(2, gp),
                lambda: x_dma(1, 0, sy),
                lambda: x_dma(1, 1, sc),
                lambda: x_dma(1, 2, gp),
                lambda: x_dma(1, 3, sy),
                lambda: x_dma(1, 4, sc),
                lambda: x_dma(1, 5, gp),
                lambda: wo_dma(0, sy),
                lambda: wo_dma(1, sc),
                lambda: wo_dma(2, gp),
                lambda: wo_dma(3, sy),
                lambda: wo_dma(4, sc),
                lambda: wo_dma(5, gp),
                lambda: sc.dma_start(out=bo_t[:], in_=bo_d.ap()),
            ]
            for emit in plan:
                emit()

            # ---- building blocks ----
            qk_store = {}

            def qk_chunk(b, ob):
                def emit():
                    ps = pj_ps.tile([128, S], f32, tag="pj", name="pj_ps_t")
                    xt = x_t[b]
                    for j, k in enumerate(KORD):
                        nc.tensor.matmul(
                            ps[:],
                            lhsT=wqk_t[ob][:, 128 * k:128 * (k + 1)],
                            rhs=xt[k][:],
                            start=(j == 0), stop=(j == KC - 1),
                        )
                    t = qk_p.tile([128, S], cdt, tag=f"qk{ob}", name=f"qk{b}_{ob}")
                    nc.vector.tensor_scalar_add(out=t[:], in0=ps[:],
                                                scalar1=bqk_t[:, ob:ob + 1])
                    qk_store[(b, ob)] = t
                return emit

            v_store = {}

            def v_chunk(b, sb, og):
                def emit():
                    if og == 0:
                        vt = v_p.tile([128, NH * VW], cdt, tag=f"v{sb}", name=f"v{b}_{sb}")
                        # upper 64 cols of each head slot must be 1.0 (the
                        # softmax-denominator columns); set the whole tile and
                        # let the copies below overwrite the v columns
                        nc.gpsimd.memset(vt[:], 1.0)
                        v_store[(b, sb)] = vt
                    vt = v_store[(b, sb)]
                    o0, w = (0, 512) if og == 0 else (512, 256)
                    xt = x_t[b]
                    ps = pj_ps.tile([128, S], f32, tag="pj", name="pj_ps_t")
                    for k in range(KC):
                        nc.tensor.matmul(
                            ps[:, :w],
                            lhsT=xt[k][:, 128 * sb:128 * (sb + 1)],
                            rhs=wv_t[:, H * k + o0:H * k + o0 + w],
                            start=(k == 0), stop=False,
                        )
                    nc.tensor.matmul(
                        ps[:, :w],
                        lhsT=on_t[:],
                        rhs=bv_t[:, o0:o0 + w],
                        start=False, stop=True,
                    )
                    nh = w // DK
                    h0 = o0 // DK
                    src = ps[:, :w].rearrange("p (h c) -> p h c", h=nh)
                    dst = vt[:].rearrange("p (h c) -> p h c", h=NH)[:, h0:h0 + nh, 0:DK]
                    nc.vector.tensor_copy(out=dst, in_=src)
                return emit

            out_q = [0]

            def fproj(b, sb, o0, w, yb_list, engs, act_copy=False):
                st = {}

                def emit_a():
                    # bias matmul first (start=True clears PSUM, and it has
                    # no dependency on the last head pair), then heads 0-4
                    ps = pj_ps.tile([128, 512], f32, tag="pj", name="pj_ps_t")
                    st["ps"] = ps
                    nc.tensor.matmul(
                        ps[:, :w],
                        lhsT=on_t[:],
                        rhs=bo_t[:, o0:o0 + w],
                        start=True, stop=False,
                    )
                    for hb in range(KC - 1):
                        nc.tensor.matmul(
                            ps[:, :w],
                            lhsT=yb_list[hb][:, 128 * sb:128 * (sb + 1)],
                            rhs=wo_t[hb][:, o0:o0 + w],
                            start=False, stop=False,
                        )

                def emit_b():
                    ps = st["ps"]
                    nc.tensor.matmul(
                        ps[:, :w],
                        lhsT=yb_list[KC - 1][:, 128 * sb:128 * (sb + 1)],
                        rhs=wo_t[KC - 1][:, o0:o0 + w],
                        start=False, stop=True,
                    )
                    ot = tm_p.tile([128, 512], f32, tag="ot", name="ot")
                    if act_copy:
                        nc.scalar.copy(out=ot[:, :w], in_=ps[:, :w])
                    else:
                        nc.vector.tensor_copy(out=ot[:, :w], in_=ps[:, :w])
                    eng = engs[out_q[0] % len(engs)]
                    out_q[0] += 1
                    eng.dma_start(
                        out=out_d.ap()[b, 128 * sb:128 * (sb + 1), o0:o0 + w],
                        in_=ot[:, :w],
                    )
                return emit_a, emit_b

            def attention(b, pending, late_pending=None, yb_out=None):
                """Head-pair attention for batch b; pops `pending` PE-filler
                closures into the ACT-bound gaps. `late_pending` items may
                depend on pairs 0..4 of this batch, so they only pop during
                the last pair (after pair 4 is emitted)."""
                late_pending = late_pending or []
                yb_t = [yb_p.tile([128, S], cdt, tag=f"yb{hb}", name=f"yb{b}_{hb}")
                        for hb in range(KC)]
                if yb_out is not None:
                    yb_out.extend(yb_t)
                for hp in range(NH // 2):
                    live = late_pending if (hp == NH // 2 - 1 and late_pending) \
                        else pending
                    pair = (2 * hp, 2 * hp + 1)
                    q_tile = qk_store[(b, hp)]
                    k_tile = qk_store[(b, KC + hp)]
                    pts = {h: [] for h in pair}
                    for kb in range(SBLK):
                        scp = sc_ps.tile([128, 2 * S], f32, tag="sc", name="sc_ps_t")
                        for hi, h in enumerate(pair):
                            krow = (h % 2) * DK
                            nc.tensor.matmul(
                                scp[:, hi * S:(hi + 1) * S],
                                lhsT=k_tile[krow:krow + DK, 128 * kb:128 * (kb + 1)],
                                rhs=q_tile[krow:krow + DK, :],
                                start=True, stop=True,
                            )
                        ptt = pt_p.tile([128, 2 * S], cdt, tag="ptt", name="ptt")
                        nc.scalar.activation(out=ptt[:], in_=scp[:], func=EXP,
                                             scale=float(1.0 / np.sqrt(DK)))
                        for hi, h in enumerate(pair):
                            pts[h].append(ptt[:, hi * S:(hi + 1) * S])
                        if kb in (1, 3) and live:
                            live.pop(0)()
                    yps = {h: ya_ps.tile([128, S], f32, tag="ya", name="ya_ps_t")
                           for h in pair}
                    for kb in range(SBLK):
                        for h in pair:
                            nc.tensor.matmul(
                                yps[h][:],
                                lhsT=v_store[(b, kb)][:, VW * h:VW * (h + 1)],
                                rhs=pts[h][kb][:],
                                start=(kb == 0), stop=(kb == SBLK - 1),
                            )
                    if live:
                        live.pop(0)()
                    # PSUM rows 64..127 of each head's AV tile hold the
                    # softmax denominator broadcast across 64 partitions
                    den = rc_p.tile([128, S], f32, tag="rec0", name="den")
                    for hi, h in enumerate(pair):
                        nc.vector.tensor_copy(out=den[hi * DK:(hi + 1) * DK, :],
                                              in_=yps[h][DK:2 * DK, :])
                    rec = rc_p.tile([128, S], f32, tag="rec1", name="rec")
                    nc.vector.reciprocal_approx_fast(out=rec[:], in_=den[:])
                    for hi, h in enumerate(pair):
                        krow = hi * DK
                        nc.vector.tensor_mul(out=yb_t[hp][krow:krow + DK, :],
                                             in0=yps[h][0:DK, :],
                                             in1=rec[krow:krow + DK, :])
                while pending:
                    pending.pop(0)()
                while late_pending:
                    late_pending.pop(0)()
                return yb_t

            # ---- batch 0: QKV projection (DMA-paced head phase) ----
            for ob in range(2 * KC):
                qk_chunk(0, ob)()
            for sb in range(SBLK):
                for og in range(2):
                    v_chunk(0, sb, og)()

            # ---- attention(0), filled with QKV(1); defer batch 1's last
            # head-pair q/k blocks into attention(1) for ACT/PE balance ----
            pend0 = []
            for ob in range(2 * KC):
                if ob in (KC - 1, 2 * KC - 1):
                    continue
                pend0.append(qk_chunk(1, ob))
            for sb in range(SBLK):
                for og in range(2):
                    pend0.append(v_chunk(1, sb, og))
            yb0 = attention(0, pend0)

            # ---- attention(1), filled with deferred q/k blocks + fproj(0);
            # the first two drain chunks' A passes ride along at the end so
            # the PE stays busy (and HAM warm) through the last pair's
            # normalize chain ----
            yb1_holder = []
            drain = []
            for sb in range(SBLK):
                for (o0, w) in ((0, 512), (512, 256)):
                    drain.append(fproj(1, sb, o0, w, yb1_holder,
                                       engs=[nc.sync, nc.scalar],
                                       act_copy=(len(drain) % 2 == 0)))

            pend1 = [qk_chunk(1, KC - 1), qk_chunk(1, 2 * KC - 1)]
            for sb in range(SBLK):
                for (o0, w) in ((0, 512), (512, 256)):
                    ea, eb = fproj(0, sb, o0, w, yb0, engs=[nc.sync, nc.gpsimd])
                    pend1.append(lambda ea=ea, eb=eb: (ea(), eb()))
            attention(1, pend1, late_pending=[drain[0][0], drain[1][0]],
                      yb_out=yb1_holder)

            # ---- fproj(1) drain: pass B (head 5 + store) interleaved with
            # the next chunks' pass A at PSUM pipeline depth 2 (A0/A1 already
            # ran as late fillers inside attention(1)) ----
            for c in range(len(drain)):
                drain[c][1]()
                if c + 2 < len(drain):
                    drain[c + 2][0]()

    nc.compile()
    return nc


def get_program():
    if "nc" not in _PROG_CACHE:
        _PROG_CACHE["nc"] = _build_program()
    return _PROG_CACHE["nc"]


def make_in_maps(x, w_qkv_w, w_qkv_b, w_o_w, w_o_b):
    import ml_dtypes
    np_cdt = ml_dtypes.bfloat16
    x = np.asarray(x, np.float32)
    xT = np.ascontiguousarray(np.transpose(x, (0, 2, 1)).astype(np_cdt))  # [B, H, S]
    wqkvT = np.asarray(w_qkv_w, np.float32).T  # [H, 3H]
    # q,k columns blocked per transposed output block:
    # wqkb[ob][p][k*128+c] = wqkvT[k*128+p, ob*128+c]
    t = wqkvT[:, :2 * H].reshape(KC, 128, 2 * KC, 128)
    wqkb = np.ascontiguousarray(t.transpose(2, 1, 0, 3).reshape(2 * KC, 128, KC * 128).astype(np_cdt))
    # v columns in contraction-chunk-major rows: wvb[p][k*H+c] = wqkvT[k*128+p, 2H+c]
    tv = wqkvT[:, 2 * H:].reshape(KC, 128, H)
    wvb = np.ascontiguousarray(tv.transpose(1, 0, 2).reshape(128, KC * H).astype(np_cdt))
    woT = np.ascontiguousarray(np.asarray(w_o_w, np.float32).T.astype(np_cdt))  # [H, H]
    # qk bias as [128, 12] f32: bqk[p, j] = w_qkv_b[j*128+p]
    bqk = np.ascontiguousarray(
        np.asarray(w_qkv_b, np.float32)[:2 * H].reshape(2 * KC, 128).T)
    bv = np.ascontiguousarray(np.asarray(w_qkv_b, np.float32)[2 * H:].reshape(1, H).astype(np_cdt))
    bo = np.ascontiguousarray(np.asarray(w_o_b, np.float32).reshape(1, H).astype(np_cdt))
    return [
        {
            "xt": np.ascontiguousarray(xT[NB * c:NB * (c + 1)]),
            "wqkb": wqkb,
            "wvb": wvb,
            "wot": woT,
            "bqk": bqk,
            "bv": bv,
            "bo": bo,
        }
        for c in range(N_CORES)
    ]


def _numpy_fallback(x, attn_mask, w_qkv_w, w_qkv_b, w_o_w, w_o_b):
    x = np.asarray(x, np.float64)
    qkv = x @ np.asarray(w_qkv_w, np.float64).T + np.asarray(w_qkv_b, np.float64)
    q, k, v = np.split(qkv, 3, axis=-1)

    def heads(t):
        return t.reshape(B, S, NH, DK).transpose(0, 2, 1, 3)

    q, k, v = heads(q), heads(k), heads(v)
    s = np.einsum("bhqd,bhkd->bhqk", q, k) / np.sqrt(DK)
    mask = np.asarray(attn_mask, bool)[:, None, None, :]
    s = np.where(mask, s, -np.inf)
    s = s - s.max(axis=-1, keepdims=True)
    p = np.exp(s)
    p = p / p.sum(axis=-1, keepdims=True)
    y = np.einsum("bhqk,bhkd->bhqd", p, v)
    y = y.transpose(0, 2, 1, 3).reshape(B, S, H)
    out = y @ np.asarray(w_o_w, np.float64).T + np.asarray(w_o_b, np.float64)
    return out.astype(np.float32)


def kernel(x, attn_mask, w_qkv_w, w_qkv_b, w_o_w, w_o_b):
    if not bool(np.all(np.asarray(attn_mask))):
        return _numpy_fallback(x, attn_mask, w_qkv_w, w_qkv_b, w_o_w, w_o_b)

    from concourse.bass_utils import run_bass_kernel_spmd

    nc = get_program()
    in_maps = make_in_maps(x, w_qkv_w, w_qkv_b, w_o_w, w_o_b)
    res = run_bass_kernel_spmd(nc, in_maps, list(range(N_CORES)))
    out = np.concatenate([res.results[c]["out"] for c in range(N_CORES)], axis=0)
    return out.astype(np.float32)
